# revision 46
# baseline (speedup 1.0000x reference)
"""Trainium2 Bass kernel for combined cross-entropy + batch-hard triplet loss.

Problem (N=4096, C=751, D=2048, 1024 identities x 4 instances):
  loss = mean(-log_softmax(logits)[i, t_i]) +
         mean(relu(max_same(dist) - min_diff(dist) + 0.5))
  with dist = pairwise Euclidean distances of feat rows.

v4 design — symmetric block assignment over 8 cores:
- feat is quantized to fp8e4m3 on the host; Gram blocks are computed with
  DoubleRow fp8 matmuls (2 K-chunks of 128 per instruction, 2x PE rate).
  sq is recomputed from the QUANTIZED features so d2 = sq_i + sq_j - 2*G
  is exactly the distance matrix of the quantized features.
- dist is symmetric: each unordered pair of 512-row blocks is computed
  once. Core c computes blocks [c] x [c, c+1, c+2, c+3, c+4] (mod 8).
  Distance d=1..3 pairs are owned by one core; the d=4 pair is computed
  by both cores of the pair (128 extra matmuls total, simpler than a
  half-split and keeps the program uniform across cores).
- Core c's fTq input has its 5 column blocks laid out locally (own block
  first), so the program is identical on every core; the own-rows weight
  tiles are slices of the same resident tensor (no separate lhsT input).
- A fold matmul adds -sq_i/2 - sq_j/2 (so psum = -d2/2, symmetric) and,
  on the diagonal block, -65536 on same-identity pairs (rows pre-sorted
  by target on the host -> 4-row groups at fixed local positions).
- Mining: row max of psum over the 5 direct blocks (hardest negative
  for own rows); row min over the m-tile's 128-col diagonal window
  (hardest positive; the -65536 mask guarantees masked entries win).
  For blocks 1..4 the TRANSPOSED direction (candidates for the other
  block's rows) is mined with one DVE tensor_reduce(apply_transpose)
  per psum tile: a 32x32 block transpose + X-reduce gives per-32-row-
  slab column maxes, which the host max-combines.
- The sqrt/relu/margin tail and cross-core an merge run on the host.
- Cross entropy: logits in bf16; device computes row max and
  sum(exp(l - max)) via ACT Exp with fused accumulation; host does ln
  and the target-logit gather.

Per-core outputs:
  out  [128, 16]: cols 0..3 row-max(psum, 5 blocks) per m; 4..7 row-min
                  (diag window) per m; 8..11 logits max; 12..15 exp sums
  out2 [128, 256]: col ((m*4)+(b-1))*16 + cb holds, at partition
                  bp*32+r, max over rows [bp*32,bp*32+32) of m-tile m of
                  psum block b, column cb*32+r  (b = 1..4)
"""

import os
import sys

if "/opt/trn_rl_repo" not in sys.path:
    sys.path.insert(0, "/opt/trn_rl_repo")

import numpy as np
import ml_dtypes

N = 4096
D = 2048
C = 751
NCORES = 8
RPC = N // NCORES          # rows per core = 512
MT = RPC // 128            # 128-row tiles per core = 4
NBLK = 5                   # column blocks per core (own + 4 neighbors)
LOC = NBLK * 512           # local columns = 2560
KT = D // 128              # 128-row contraction chunks = 16
KF = 36                    # fold contraction: 2 sq_j + 2 sq_i + 32 mask
BIG = 131072.0             # 2^17 offset on same pairs in d2 = -2*psum
MASK_SCALE = 256.0         # 2^8, exact in bf16
ALPHA = 1.0
BETA = 1.0
MARGIN = 0.5

GRAM_MODE = os.environ.get("GRAM_MODE", "fp8")   # "fp8" | "bf16"

_compiled = {}


def _build_nc():
    import concourse.bass as bass  # noqa: F401
    import concourse.tile as tile
    from concourse import mybir, bacc
    from contextlib import ExitStack

    f32 = mybir.dt.float32
    bf16 = mybir.dt.bfloat16
    f8 = mybir.dt.float8e4
    gdt = f8 if GRAM_MODE == "fp8" else bf16
    Alu = mybir.AluOpType
    Act = mybir.ActivationFunctionType
    X = mybir.AxisListType.X
    XY = mybir.AxisListType.XY
    DR = mybir.MatmulPerfMode.DoubleRow if GRAM_MODE == "fp8" else None

    nc = bacc.Bacc("TRN2", target_bir_lowering=False, debug=False)

    fTq_in = nc.dram_tensor("fTq", [D, LOC], gdt, kind="ExternalInput").ap()
    frd_in = nc.dram_tensor("fold_diag", [KF, MT * 512], bf16, kind="ExternalInput").ap()
    fro_in = nc.dram_tensor("fold_off", [4, LOC], bf16, kind="ExternalInput").ap()
    flh_in = nc.dram_tensor("fold_lhsT", [KF, MT * 128], bf16, kind="ExternalInput").ap()
    logits_in = nc.dram_tensor("logits", [RPC, C], bf16, kind="ExternalInput").ap()
    out_dram = nc.dram_tensor("out", [128, 16], f32, kind="ExternalOutput").ap()
    out2_dram = nc.dram_tensor("out2", [128, 256], f32, kind="ExternalOutput").ap()

    with tile.TileContext(nc) as tc, ExitStack() as ctx:
        resident = ctx.enter_context(tc.tile_pool(name="resident", bufs=1))
        psum_pool = ctx.enter_context(tc.tile_pool(name="psum", bufs=8, space="PSUM"))
        xent_pool = ctx.enter_context(tc.tile_pool(name="xent", bufs=2))
        small_pool = ctx.enter_context(tc.tile_pool(name="small", bufs=4))

        NP = KT // 2   # chunk pairs = 8
        ftp = [resident.tile([128, 2, LOC], gdt, tag=f"ftp{j}", name=f"ftp{j}")
               for j in range(NP)]
        frd = resident.tile([KF, MT * 512], bf16)
        fro = resident.tile([4, LOC], bf16)
        flh = resident.tile([KF, MT * 128], bf16)
        out_tile = resident.tile([128, 16], f32)
        out2m = [resident.tile([128, 64], f32, tag=f"o2m{m}", name=f"o2m{m}")
                 for m in range(MT)]
        lg = [resident.tile([128, C], bf16, tag=f"lg{r}", name=f"lg{r}") for r in range(MT)]
        mx = [resident.tile([128, NBLK], f32, tag=f"mx{m}", name=f"mx{m}") for m in range(MT)]

        def rhs(j, b):
            return ftp[j][:, :, bass.ts(b, 512)]

        def wslice(j, m):
            return ftp[j][:, :, bass.ts(m, 128)]

        # --- input DMAs; per-pair tiles so matmul j only waits on pair j.
        # The first two pairs go out on the Activation HWDGE queue in case
        # its sequencer clears the runtime preamble before Sync's.
        def load_pair(j):
            eng = nc.scalar if j < 2 else nc.sync
            for i in (0, 1):
                eng.dma_start(ftp[j][:, i, :], fTq_in[bass.ts(2 * j + i, 128), :])

        load_pair(0)
        load_pair(1)
        load_pair(2)
        load_pair(3)
        nc.sync.dma_start(flh[:], flh_in[:])
        nc.sync.dma_start(frd[:], frd_in[:])
        nc.sync.dma_start(fro[:], fro_in[:])
        for j in range(4, NP):
            load_pair(j)
        for r in range(MT):
            nc.sync.dma_start(lg[r][:], logits_in[bass.ts(r, 128), :])

        if GRAM_MODE == "fp8":
            # b4/m>=2 quadrant split leaves out2 cols 48..55 unwritten
            for m in (2, 3):
                nc.vector.memset(out2m[m][:, 48:56], 0.0)
            # NOTE: a memset-backed PE warmup chain (16 production-shaped
            # dummy matmuls bridging the runtime preamble) was measured
            # neutral: the real stream starts fully warm, but the drain
            # handoff costs what the cold-clock phase saved. Removed for
            # simplicity.

        # --- Gram + fold + mining ---
        for m in range(MT):
            pss = [psum_pool.tile([128, 16, 32], f32, tag="ps", name=f"ps{m}_{b}")
                   for b in range(NBLK)]
            half4 = GRAM_MODE == "fp8" and m >= 2

            def gram_mm(j, b):
                w = wslice(j, m)
                if b == 4 and half4:
                    # block c+4 quadrant split: m-tiles 2,3 only compute
                    # cols 256..511 (the partner core's m-tiles 0,1 cover
                    # the rest via transpose)
                    nc.tensor.matmul(
                        pss[4][:, 8:16, :], w,
                        ftp[j][:, :, bass.ds(4 * 512 + 256, 256)],
                        start=(j == 0), stop=False, perf_mode=DR)
                else:
                    nc.tensor.matmul(pss[b][:], w, rhs(j, b),
                                     start=(j == 0), stop=False, perf_mode=DR)

            def fold_mm(b):
                # diag block (local b=0) gets sq both sides + same-mask
                # (K=36); others sq both sides only (K=4)
                if b == 0:
                    nc.tensor.matmul(pss[0][:], flh[:, bass.ts(m, 128)],
                                     frd[:, bass.ts(m, 512)], start=False, stop=True)
                elif b == 4 and half4:
                    nc.tensor.matmul(pss[4][:, 8:16, :],
                                     flh[0:4, bass.ts(m, 128)],
                                     fro[:, bass.ds(4 * 512 + 256, 256)],
                                     start=False, stop=True)
                else:
                    nc.tensor.matmul(pss[b][:], flh[0:4, bass.ts(m, 128)],
                                     fro[:, bass.ts(b, 512)], start=False, stop=True)

            def mine(b):
                # transposed direction first (32x32 block transpose +
                # X-reduce = per-slab column maxes for the partner block's
                # rows), then the direct row max
                pb = pss[b][:, 8:16, :] if (b == 4 and half4) else pss[b][:]
                if b >= 1:
                    wd = 8 if (b == 4 and half4) else 16
                    col = (b - 1) * 16 + (8 if (b == 4 and half4) else 0)
                    nc.vector.tensor_reduce(out2m[m][:, col:col + wd],
                                            pb, axis=X, op=Alu.max,
                                            apply_transpose=True)
                nc.vector.tensor_reduce(mx[m][:, b:b + 1], pb, axis=XY,
                                        op=Alu.max)
                if b == 0:
                    nc.vector.tensor_reduce(out_tile[:, 4 + m:5 + m],
                                            pss[0][:, bass.ds(m * 4, 4), :],
                                            axis=XY, op=Alu.min)

            if GRAM_MODE == "fp8":
                for j in range(NP):
                    for b in range(NBLK):
                        gram_mm(j, b)
                for b in range(NBLK):
                    fold_mm(b)
                for b in range(NBLK):
                    mine(b)
            else:
                for j in range(NP):
                    for i in (0, 1):
                        w = wslice(j, m)[:, i, :]
                        for b in range(NBLK):
                            nc.tensor.matmul(
                                pss[b][:], w, rhs(j, b)[:, i, :],
                                start=(j == 0 and i == 0), stop=False,
                            )
                for b in range(NBLK):
                    fold_mm(b)
                for b in range(NBLK):
                    mine(b)
            nc.sync.dma_start(out2_dram[:, bass.ts(m, 64)], out2m[m][:])
            nc.vector.tensor_reduce(out_tile[:, m:m + 1], mx[m][:], axis=X, op=Alu.max)

            if m == 0:
                # xent: DVE row-max + negate, then ACT exp with accumulation
                negs = []
                for r in range(MT):
                    nc.vector.tensor_reduce(out_tile[:, 8 + r:9 + r], lg[r][:],
                                            axis=X, op=Alu.max)
                    neg = small_pool.tile([128, 1], f32, tag=f"neg{r}", name=f"neg{r}")
                    nc.vector.tensor_scalar_mul(neg[:], out_tile[:, 8 + r:9 + r], -1.0)
                    negs.append(neg)
                for r in range(MT):
                    escr = xent_pool.tile([128, C], bf16, tag="escr", name=f"escr{r}")
                    nc.scalar.activation(escr[:], lg[r][:], Act.Exp,
                                         bias=negs[r][:], scale=1.0,
                                         accum_out=out_tile[:, 12 + r:13 + r])

        nc.sync.dma_start(out_dram[:], out_tile[:])

    nc.compile()
    return nc


def _prepare(logits, feat, targets):
    logits = np.asarray(logits, dtype=np.float32)
    feat = np.asarray(feat, dtype=np.float32)
    targets = np.asarray(targets)

    perm = np.argsort(targets, kind="stable")
    t = np.asarray(targets)[perm]
    tg = t.reshape(-1, 4)
    assert (tg == tg[:, :1]).all(), "expected PK sampling with 4 instances/identity"

    feat_p = feat[perm]
    logits_p = logits[perm]

    gdt = ml_dtypes.float8_e4m3 if GRAM_MODE == "fp8" else ml_dtypes.bfloat16
    fq_small = feat_p.astype(gdt)                       # quantized [N, D]
    fq = fq_small.astype(np.float64)
    fTq = np.ascontiguousarray(fq_small.T)              # [D, N]
    sq = np.einsum("ij,ij->i", fq, fq).astype(np.float32)

    hi = sq.astype(ml_dtypes.bfloat16)
    lo = (sq.astype(np.float64) - hi.astype(np.float64)).astype(ml_dtypes.bfloat16)
    row_hi = (-0.5 * hi.astype(np.float32)).astype(ml_dtypes.bfloat16)
    row_lo = (-0.5 * lo.astype(np.float32)).astype(ml_dtypes.bfloat16)

    lgq = logits_p.astype(ml_dtypes.bfloat16)

    # target logit (host gather, matching jax clamp semantics)
    ti = t.astype(np.int64)
    ti = np.where(ti < 0, ti + C, ti)
    ti = np.clip(ti, 0, C - 1)
    tlog = logits_p[np.arange(N), ti].astype(np.float64)

    # diag fold mask rows (identical on every core; local positions)
    frd_mask = np.zeros((KF, MT, 512), dtype=ml_dtypes.bfloat16)
    for m in range(MT):
        for g in range(32):
            frd_mask[4 + g, m, m * 128 + 4 * g: m * 128 + 4 * g + 4] = -MASK_SCALE

    # fold lhsT mask rows (row 4+g has 256 at cols 4g..4g+3 of each m slice)
    cols = np.arange(128)

    in_maps = []
    for c in range(NCORES):
        rows = slice(c * RPC, (c + 1) * RPC)
        blocks = [(c + b) % NCORES for b in range(NBLK)]
        loc = np.concatenate([np.arange(a * 512, (a + 1) * 512) for a in blocks])

        frd = frd_mask.copy()
        frd[0, :, :] = row_hi[rows][None, :]
        frd[1, :, :] = row_lo[rows][None, :]
        frd[2, :, :] = 1.0
        frd[3, :, :] = 1.0

        fro = np.zeros((4, LOC), dtype=ml_dtypes.bfloat16)
        fro[0] = row_hi[loc]
        fro[1] = row_lo[loc]
        fro[2] = 1.0
        fro[3] = 1.0

        flh = np.zeros((KF, MT, 128), dtype=ml_dtypes.bfloat16)
        flh[0] = 1.0
        flh[1] = 1.0
        for m in range(MT):
            mrows = slice(c * RPC + m * 128, c * RPC + m * 128 + 128)
            flh[2, m] = row_hi[mrows]
            flh[3, m] = row_lo[mrows]
            flh[4 + cols // 4, m, cols] = MASK_SCALE

        in_maps.append({
            "fTq": np.ascontiguousarray(fTq[:, loc]),
            "fold_diag": np.ascontiguousarray(frd.reshape(KF, MT * 512)),
            "fold_off": fro,
            "fold_lhsT": np.ascontiguousarray(flh.reshape(KF, MT * 128)),
            "logits": np.ascontiguousarray(lgq[rows]),
        })
    return in_maps, sq, tlog


def _combine(results, sq, tlog):
    outs = np.stack([r["out"].astype(np.float64) for r in results])   # [8, 128, 16]
    out2 = np.stack([r["out2"].astype(np.float64) for r in results])  # [8, 128, 256]

    # direct per-row max/min: global row (c, m, p) -> c*512 + m*128 + p
    mx = outs[:, :, 0:4].transpose(0, 2, 1).reshape(N)       # row max psum (5 blocks)
    mn = outs[:, :, 4:8].transpose(0, 2, 1).reshape(N)       # row min diag window
    lmx = outs[:, :, 8:12].transpose(0, 2, 1).reshape(N)     # logits max
    les = outs[:, :, 12:16].transpose(0, 2, 1).reshape(N)    # exp sums

    # transposed contributions: out2[c][bp*32+r, ((m*4)+(b-1))*16+cb] =
    # max over rows of (c,m) slab bp of psum block b, local col cb*32+r.
    # Reshape to [core, bp, r, m, b, cb] then max over (bp, m) ->
    # colmax[c, b, q_loc] with q_loc = cb*32 + r.
    g = out2.reshape(NCORES, 4, 32, MT, 4, 16)      # [c, bp, r, m, b-1, cb]
    if GRAM_MODE == "fp8":
        # b4 quadrant split: m-tiles 2,3 only computed cols 256..511
        g[:, :, :, 2:, 3, :8] = -np.inf
    colmax = g.max(axis=(1, 3))                     # [c, r, b-1, cb]
    colmax = colmax.transpose(0, 2, 3, 1)           # [c, b-1, cb, r]
    colmax = colmax.reshape(NCORES, 4, 512)         # [c, b-1, q_loc]

    anmax = mx.copy()
    for b in range(1, NBLK):
        for c in range(NCORES):
            a = (c + b) % NCORES                    # partner block
            tgt = slice(a * 512, (a + 1) * 512)
            anmax[tgt] = np.maximum(anmax[tgt], colmax[c, b - 1])

    an2 = np.maximum(-2.0 * anmax, 1e-12)
    ap2 = np.maximum(-2.0 * mn - BIG, 1e-12)
    trip = np.maximum(np.sqrt(ap2) - np.sqrt(an2) + MARGIN, 0.0)

    lse = lmx + np.log(les)
    xent = lse - tlog

    loss = ALPHA * xent.mean() + BETA * trip.mean()
    return np.float32(loss)


def kernel(logits, feat, targets):
    from concourse.bass_utils import run_bass_kernel_spmd

    if "nc" not in _compiled:
        _compiled["nc"] = _build_nc()
    nc = _compiled["nc"]

    in_maps, sq, tlog = _prepare(logits, feat, targets)
    res = run_bass_kernel_spmd(nc, in_maps, core_ids=list(range(NCORES)))
    return _combine(res.results, sq, tlog)


# revision 47
# speedup vs baseline: 1.0850x; 1.0850x over previous
"""Trainium2 Bass kernel for combined cross-entropy + batch-hard triplet loss.

Problem (N=4096, C=751, D=2048, 1024 identities x 4 instances):
  loss = mean(-log_softmax(logits)[i, t_i]) +
         mean(relu(max_same(dist) - min_diff(dist) + 0.5))
  with dist = pairwise Euclidean distances of feat rows.

v4 design — symmetric block assignment over 8 cores:
- feat is quantized to fp8e4m3 on the host; Gram blocks are computed with
  DoubleRow fp8 matmuls (2 K-chunks of 128 per instruction, 2x PE rate).
  sq is recomputed from the QUANTIZED features so d2 = sq_i + sq_j - 2*G
  is exactly the distance matrix of the quantized features.
- dist is symmetric: each unordered pair of 512-row blocks is computed
  once. Core c computes blocks [c] x [c, c+1, c+2, c+3, c+4] (mod 8).
  Distance d=1..3 pairs are owned by one core; the d=4 pair is computed
  by both cores of the pair (128 extra matmuls total, simpler than a
  half-split and keeps the program uniform across cores).
- Core c's fTq input has its 5 column blocks laid out locally (own block
  first), so the program is identical on every core; the own-rows weight
  tiles are slices of the same resident tensor (no separate lhsT input).
- A fold matmul adds -sq_i/2 - sq_j/2 (so psum = -d2/2, symmetric) and,
  on the diagonal block, -65536 on same-identity pairs (rows pre-sorted
  by target on the host -> 4-row groups at fixed local positions).
- Mining: row max of psum over the 5 direct blocks (hardest negative
  for own rows); row min over the m-tile's 128-col diagonal window
  (hardest positive; the -65536 mask guarantees masked entries win).
  For blocks 1..4 the TRANSPOSED direction (candidates for the other
  block's rows) is mined with one DVE tensor_reduce(apply_transpose)
  per psum tile: a 32x32 block transpose + X-reduce gives per-32-row-
  slab column maxes, which the host max-combines.
- The sqrt/relu/margin tail and cross-core an merge run on the host.
- Cross entropy: logits in bf16; device computes row max and
  sum(exp(l - max)) via ACT Exp with fused accumulation; host does ln
  and the target-logit gather.

Per-core outputs:
  out  [128, 16]: cols 0..3 row-max(psum, 5 blocks) per m; 4..7 row-min
                  (diag window) per m; 8..11 logits max; 12..15 exp sums
  out2 [128, 256]: col ((m*4)+(b-1))*16 + cb holds, at partition
                  bp*32+r, max over rows [bp*32,bp*32+32) of m-tile m of
                  psum block b, column cb*32+r  (b = 1..4)
"""

import os
import sys

if "/opt/trn_rl_repo" not in sys.path:
    sys.path.insert(0, "/opt/trn_rl_repo")

import numpy as np
import ml_dtypes

N = 4096
D = 2048
C = 751
NCORES = 8
RPC = N // NCORES          # rows per core = 512
MT = RPC // 128            # 128-row tiles per core = 4
NBLK = 5                   # column blocks per core (own + 4 neighbors)
LOC = NBLK * 512           # local columns = 2560
KT = D // 128              # 128-row contraction chunks = 16
KF = 36                    # fold contraction: 2 sq_j + 2 sq_i + 32 mask
BIG = 131072.0             # 2^17 offset on same pairs in d2 = -2*psum
MASK_SCALE = 256.0         # 2^8, exact in bf16
ALPHA = 1.0
BETA = 1.0
MARGIN = 0.5

GRAM_MODE = os.environ.get("GRAM_MODE", "fp8")   # "fp8" | "bf16"

_compiled = {}


def _build_nc():
    import concourse.bass as bass  # noqa: F401
    import concourse.tile as tile
    from concourse import mybir, bacc
    from contextlib import ExitStack

    f32 = mybir.dt.float32
    bf16 = mybir.dt.bfloat16
    f8 = mybir.dt.float8e4
    gdt = f8 if GRAM_MODE == "fp8" else bf16
    Alu = mybir.AluOpType
    Act = mybir.ActivationFunctionType
    X = mybir.AxisListType.X
    XY = mybir.AxisListType.XY
    DR = mybir.MatmulPerfMode.DoubleRow if GRAM_MODE == "fp8" else None

    nc = bacc.Bacc("TRN2", target_bir_lowering=False, debug=False)

    fTq_in = nc.dram_tensor("fTq", [D, LOC], gdt, kind="ExternalInput").ap()
    frd_in = nc.dram_tensor("fold_diag", [KF, MT * 512], bf16, kind="ExternalInput").ap()
    fro_in = nc.dram_tensor("fold_off", [4, LOC], bf16, kind="ExternalInput").ap()
    flh_in = nc.dram_tensor("fold_lhsT", [KF, MT * 128], bf16, kind="ExternalInput").ap()
    logits_in = nc.dram_tensor("logits", [RPC, C], bf16, kind="ExternalInput").ap()
    out_dram = nc.dram_tensor("out", [128, 16], f32, kind="ExternalOutput").ap()
    out2_dram = nc.dram_tensor("out2", [128, 256], f32, kind="ExternalOutput").ap()

    with tile.TileContext(nc) as tc, ExitStack() as ctx:
        resident = ctx.enter_context(tc.tile_pool(name="resident", bufs=1))
        psum_pool = ctx.enter_context(tc.tile_pool(name="psum", bufs=8, space="PSUM"))
        xent_pool = ctx.enter_context(tc.tile_pool(name="xent", bufs=2))
        small_pool = ctx.enter_context(tc.tile_pool(name="small", bufs=4))

        NP = KT // 2   # chunk pairs = 8
        ftp = [resident.tile([128, 2, LOC], gdt, tag=f"ftp{j}", name=f"ftp{j}")
               for j in range(NP)]
        frd = resident.tile([KF, MT * 512], bf16)
        fro = resident.tile([4, LOC], bf16)
        flh = resident.tile([KF, MT * 128], bf16)
        out_tile = resident.tile([128, 16], f32)
        out2m = [resident.tile([128, 64], f32, tag=f"o2m{m}", name=f"o2m{m}")
                 for m in range(MT)]
        lg = [resident.tile([128, C], bf16, tag=f"lg{r}", name=f"lg{r}") for r in range(MT)]
        mx = [resident.tile([128, NBLK], f32, tag=f"mx{m}", name=f"mx{m}") for m in range(MT)]

        def rhs(j, b):
            return ftp[j][:, :, bass.ts(b, 512)]

        def wslice(j, m):
            return ftp[j][:, :, bass.ts(m, 128)]

        # --- input DMAs; per-pair tiles so matmul j only waits on pair j ---
        def load_pair(j):
            for i in (0, 1):
                nc.sync.dma_start(ftp[j][:, i, :], fTq_in[bass.ts(2 * j + i, 128), :])

        load_pair(0)
        load_pair(1)
        load_pair(2)
        load_pair(3)
        nc.sync.dma_start(flh[:], flh_in[:])
        nc.sync.dma_start(frd[:], frd_in[:])
        nc.sync.dma_start(fro[:], fro_in[:])
        for j in range(4, NP):
            load_pair(j)
        for r in range(MT):
            nc.sync.dma_start(lg[r][:], logits_in[bass.ts(r, 128), :])

        if GRAM_MODE == "fp8":
            # b4/m>=2 quadrant split leaves out2 cols 48..55 unwritten
            for m in (2, 3):
                nc.vector.memset(out2m[m][:, 48:56], 0.0)
            # NOTE: a memset-backed PE warmup chain (16 production-shaped
            # dummy matmuls bridging the runtime preamble) was measured
            # neutral: the real stream starts fully warm, but the drain
            # handoff costs what the cold-clock phase saved. Removed for
            # simplicity.

        # --- Gram + fold + mining ---
        for m in range(MT):
            pss = [psum_pool.tile([128, 16, 32], f32, tag="ps", name=f"ps{m}_{b}")
                   for b in range(NBLK)]
            half4 = GRAM_MODE == "fp8" and m >= 2

            def gram_mm(j, b):
                w = wslice(j, m)
                if b == 4 and half4:
                    # block c+4 quadrant split: m-tiles 2,3 only compute
                    # cols 256..511 (the partner core's m-tiles 0,1 cover
                    # the rest via transpose)
                    nc.tensor.matmul(
                        pss[4][:, 8:16, :], w,
                        ftp[j][:, :, bass.ds(4 * 512 + 256, 256)],
                        start=(j == 0), stop=False, perf_mode=DR)
                else:
                    nc.tensor.matmul(pss[b][:], w, rhs(j, b),
                                     start=(j == 0), stop=False, perf_mode=DR)

            def fold_mm(b):
                # diag block (local b=0) gets sq both sides + same-mask
                # (K=36); others sq both sides only (K=4)
                if b == 0:
                    nc.tensor.matmul(pss[0][:], flh[:, bass.ts(m, 128)],
                                     frd[:, bass.ts(m, 512)], start=False, stop=True)
                elif b == 4 and half4:
                    nc.tensor.matmul(pss[4][:, 8:16, :],
                                     flh[0:4, bass.ts(m, 128)],
                                     fro[:, bass.ds(4 * 512 + 256, 256)],
                                     start=False, stop=True)
                else:
                    nc.tensor.matmul(pss[b][:], flh[0:4, bass.ts(m, 128)],
                                     fro[:, bass.ts(b, 512)], start=False, stop=True)

            def mine(b):
                # transposed direction first (32x32 block transpose +
                # X-reduce = per-slab column maxes for the partner block's
                # rows), then the direct row max
                pb = pss[b][:, 8:16, :] if (b == 4 and half4) else pss[b][:]
                if b >= 1:
                    wd = 8 if (b == 4 and half4) else 16
                    col = (b - 1) * 16 + (8 if (b == 4 and half4) else 0)
                    nc.vector.tensor_reduce(out2m[m][:, col:col + wd],
                                            pb, axis=X, op=Alu.max,
                                            apply_transpose=True)
                nc.vector.tensor_reduce(mx[m][:, b:b + 1], pb, axis=XY,
                                        op=Alu.max)
                if b == 0:
                    nc.vector.tensor_reduce(out_tile[:, 4 + m:5 + m],
                                            pss[0][:, bass.ds(m * 4, 4), :],
                                            axis=XY, op=Alu.min)

            if GRAM_MODE == "fp8":
                for j in range(NP):
                    for b in range(NBLK):
                        gram_mm(j, b)
                for b in range(NBLK):
                    fold_mm(b)
                for b in range(NBLK):
                    mine(b)
            else:
                for j in range(NP):
                    for i in (0, 1):
                        w = wslice(j, m)[:, i, :]
                        for b in range(NBLK):
                            nc.tensor.matmul(
                                pss[b][:], w, rhs(j, b)[:, i, :],
                                start=(j == 0 and i == 0), stop=False,
                            )
                for b in range(NBLK):
                    fold_mm(b)
                for b in range(NBLK):
                    mine(b)
            nc.sync.dma_start(out2_dram[:, bass.ts(m, 64)], out2m[m][:])
            nc.vector.tensor_reduce(out_tile[:, m:m + 1], mx[m][:], axis=X, op=Alu.max)

            if m == 0:
                # xent: DVE row-max + negate, then ACT exp with accumulation
                negs = []
                for r in range(MT):
                    nc.vector.tensor_reduce(out_tile[:, 8 + r:9 + r], lg[r][:],
                                            axis=X, op=Alu.max)
                    neg = small_pool.tile([128, 1], f32, tag=f"neg{r}", name=f"neg{r}")
                    nc.vector.tensor_scalar_mul(neg[:], out_tile[:, 8 + r:9 + r], -1.0)
                    negs.append(neg)
                for r in range(MT):
                    escr = xent_pool.tile([128, C], bf16, tag="escr", name=f"escr{r}")
                    nc.scalar.activation(escr[:], lg[r][:], Act.Exp,
                                         bias=negs[r][:], scale=1.0,
                                         accum_out=out_tile[:, 12 + r:13 + r])

        nc.sync.dma_start(out_dram[:], out_tile[:])

    nc.compile()
    return nc


def _prepare(logits, feat, targets):
    logits = np.asarray(logits, dtype=np.float32)
    feat = np.asarray(feat, dtype=np.float32)
    targets = np.asarray(targets)

    perm = np.argsort(targets, kind="stable")
    t = np.asarray(targets)[perm]
    tg = t.reshape(-1, 4)
    assert (tg == tg[:, :1]).all(), "expected PK sampling with 4 instances/identity"

    feat_p = feat[perm]
    logits_p = logits[perm]

    gdt = ml_dtypes.float8_e4m3 if GRAM_MODE == "fp8" else ml_dtypes.bfloat16
    fq_small = feat_p.astype(gdt)                       # quantized [N, D]
    fq = fq_small.astype(np.float64)
    fTq = np.ascontiguousarray(fq_small.T)              # [D, N]
    sq = np.einsum("ij,ij->i", fq, fq).astype(np.float32)

    hi = sq.astype(ml_dtypes.bfloat16)
    lo = (sq.astype(np.float64) - hi.astype(np.float64)).astype(ml_dtypes.bfloat16)
    row_hi = (-0.5 * hi.astype(np.float32)).astype(ml_dtypes.bfloat16)
    row_lo = (-0.5 * lo.astype(np.float32)).astype(ml_dtypes.bfloat16)

    lgq = logits_p.astype(ml_dtypes.bfloat16)

    # target logit (host gather, matching jax clamp semantics)
    ti = t.astype(np.int64)
    ti = np.where(ti < 0, ti + C, ti)
    ti = np.clip(ti, 0, C - 1)
    tlog = logits_p[np.arange(N), ti].astype(np.float64)

    # diag fold mask rows (identical on every core; local positions)
    frd_mask = np.zeros((KF, MT, 512), dtype=ml_dtypes.bfloat16)
    for m in range(MT):
        for g in range(32):
            frd_mask[4 + g, m, m * 128 + 4 * g: m * 128 + 4 * g + 4] = -MASK_SCALE

    # fold lhsT mask rows (row 4+g has 256 at cols 4g..4g+3 of each m slice)
    cols = np.arange(128)

    in_maps = []
    for c in range(NCORES):
        rows = slice(c * RPC, (c + 1) * RPC)
        blocks = [(c + b) % NCORES for b in range(NBLK)]
        loc = np.concatenate([np.arange(a * 512, (a + 1) * 512) for a in blocks])

        frd = frd_mask.copy()
        frd[0, :, :] = row_hi[rows][None, :]
        frd[1, :, :] = row_lo[rows][None, :]
        frd[2, :, :] = 1.0
        frd[3, :, :] = 1.0

        fro = np.zeros((4, LOC), dtype=ml_dtypes.bfloat16)
        fro[0] = row_hi[loc]
        fro[1] = row_lo[loc]
        fro[2] = 1.0
        fro[3] = 1.0

        flh = np.zeros((KF, MT, 128), dtype=ml_dtypes.bfloat16)
        flh[0] = 1.0
        flh[1] = 1.0
        for m in range(MT):
            mrows = slice(c * RPC + m * 128, c * RPC + m * 128 + 128)
            flh[2, m] = row_hi[mrows]
            flh[3, m] = row_lo[mrows]
            flh[4 + cols // 4, m, cols] = MASK_SCALE

        in_maps.append({
            "fTq": np.ascontiguousarray(fTq[:, loc]),
            "fold_diag": np.ascontiguousarray(frd.reshape(KF, MT * 512)),
            "fold_off": fro,
            "fold_lhsT": np.ascontiguousarray(flh.reshape(KF, MT * 128)),
            "logits": np.ascontiguousarray(lgq[rows]),
        })
    return in_maps, sq, tlog


def _combine(results, sq, tlog):
    outs = np.stack([r["out"].astype(np.float64) for r in results])   # [8, 128, 16]
    out2 = np.stack([r["out2"].astype(np.float64) for r in results])  # [8, 128, 256]

    # direct per-row max/min: global row (c, m, p) -> c*512 + m*128 + p
    mx = outs[:, :, 0:4].transpose(0, 2, 1).reshape(N)       # row max psum (5 blocks)
    mn = outs[:, :, 4:8].transpose(0, 2, 1).reshape(N)       # row min diag window
    lmx = outs[:, :, 8:12].transpose(0, 2, 1).reshape(N)     # logits max
    les = outs[:, :, 12:16].transpose(0, 2, 1).reshape(N)    # exp sums

    # transposed contributions: out2[c][bp*32+r, ((m*4)+(b-1))*16+cb] =
    # max over rows of (c,m) slab bp of psum block b, local col cb*32+r.
    # Reshape to [core, bp, r, m, b, cb] then max over (bp, m) ->
    # colmax[c, b, q_loc] with q_loc = cb*32 + r.
    g = out2.reshape(NCORES, 4, 32, MT, 4, 16)      # [c, bp, r, m, b-1, cb]
    if GRAM_MODE == "fp8":
        # b4 quadrant split: m-tiles 2,3 only computed cols 256..511
        g[:, :, :, 2:, 3, :8] = -np.inf
    colmax = g.max(axis=(1, 3))                     # [c, r, b-1, cb]
    colmax = colmax.transpose(0, 2, 3, 1)           # [c, b-1, cb, r]
    colmax = colmax.reshape(NCORES, 4, 512)         # [c, b-1, q_loc]

    anmax = mx.copy()
    for b in range(1, NBLK):
        for c in range(NCORES):
            a = (c + b) % NCORES                    # partner block
            tgt = slice(a * 512, (a + 1) * 512)
            anmax[tgt] = np.maximum(anmax[tgt], colmax[c, b - 1])

    an2 = np.maximum(-2.0 * anmax, 1e-12)
    ap2 = np.maximum(-2.0 * mn - BIG, 1e-12)
    trip = np.maximum(np.sqrt(ap2) - np.sqrt(an2) + MARGIN, 0.0)

    lse = lmx + np.log(les)
    xent = lse - tlog

    loss = ALPHA * xent.mean() + BETA * trip.mean()
    return np.float32(loss)


def kernel(logits, feat, targets):
    from concourse.bass_utils import run_bass_kernel_spmd

    if "nc" not in _compiled:
        _compiled["nc"] = _build_nc()
    nc = _compiled["nc"]

    in_maps, sq, tlog = _prepare(logits, feat, targets)
    res = run_bass_kernel_spmd(nc, in_maps, core_ids=list(range(NCORES)))
    return _combine(res.results, sq, tlog)
